# revision 35
# baseline (speedup 1.0000x reference)
"""AttentionNCF distributed Bass kernel for 8 TRN2 NeuronCores.

Data-parallel over B=2048 (256 rows per core); rated_items and all
weights replicated.

Math note: attention scores are a rank-1 outer sum
    s[b,i] = (cand@wc)[b] + (rated@wr)[i] + att_b
and softmax over i is shift-invariant, so the cand/bias terms cancel:
    att[b,i]*um[b,i] = um[b,i] * e[i] / S[b]
with e = exp(rated@wr) and S[b] = sum_i 1[um[b,i]!=0] * e[i].
Since nonzero ratings are >= 0.5, the mask is min(2*um, 1), and
mask*e = min(2*e*um, e).

On-chip layout: activations are kept transposed ([features, batch]),
so every Linear is matmul(lhsT=W_natural, rhs=act_T) with per-partition
biases fused into the PSUM-draining relu (alternating ScalarE/VectorE).
Matmuls run in bf16 (inputs cast on-chip from the f32 DMA stream): FWL
halves the weight-load time and LDWEIGHTS pipelines with the matmul
stream, unlike fp32/fp32r whose 4-byte weight load is fused + serial.

The kernel is DMA-bound (~37MB/core): DMA emission order is the
consumption order (rated/um groups, then item weights, then
uw1/uw2/mw1..mw4), and the weight pool is deep enough that the DMA
queues never starve while towers catch up.
"""

import os

import numpy as np

import concourse.bacc as bacc
import concourse.mybir as mybir
import concourse.tile as tile
from concourse.bass import ts
from concourse.bass_utils import run_bass_kernel_spmd

F32 = mybir.dt.float32
BF16 = mybir.dt.bfloat16
AF = mybir.ActivationFunctionType
ALU = mybir.AluOpType

NCORES = 8
B, I, D = 2048, 4096, 512
BL = B // NCORES          # 256 batch rows per core
KT = I // 128             # 32 attention k-tiles
GRP = 8                   # c-tiles fetched per DMA group (i = 32p + c)
NGRP = KT // GRP

# (K, M) for the dense layers
_LAYERS = {
    "iw1": (512, 1024),
    "iw2": (1024, 512),
    "uw1": (512, 2048),
    "uw2": (2048, 1024),
    "mw1": (1536, 1024),
    "mw2": (1024, 512),
    "mw3": (512, 256),
    "mw4": (256, 1),
}
# k-subtiles per DMA chunk (keeps chunks <= 1MB)
_KSUB = {"iw1": 2, "iw2": 4, "uw1": 1, "uw2": 2, "mw1": 2, "mw2": 2,
         "mw3": 4, "mw4": 2}

_CACHE = {}


def _build():
    nc = bacc.Bacc("TRN2", target_bir_lowering=False, debug=False)

    def param(name, shape):
        return nc.declare_dram_parameter(name, list(shape), F32,
                                         isOutput=False).ap()

    candT = param("candT", (D, BL))
    umT = param("umT", (I, BL))
    rated = param("rated", (I, D))
    wrb = param("wrb", (128, D))
    w_dram = {k: param(k, (K, M)) for k, (K, M) in _LAYERS.items()}
    # all biases merged into one [128, 51] array (see _prep_host)
    biases = param("biases", (128, 51))
    out_dram = nc.declare_dram_parameter("out", [BL], F32, isOutput=True).ap()

    WBUFS = int(os.environ.get("WBUFS", "10"))

    with tile.TileContext(nc) as tc:
        with (
            tc.tile_pool(name="const", bufs=1) as cpool,
            tc.tile_pool(name="acts", bufs=1) as apool,
            tc.tile_pool(name="stream", bufs=2) as spool,
            tc.tile_pool(name="ratedbf", bufs=2) as rbfpool,
            tc.tile_pool(name="wstage", bufs=6) as wstagepool,
            tc.tile_pool(name="wtail", bufs=1) as wtailpool,
            tc.tile_pool(name="wstream", bufs=WBUFS) as wpool,
            tc.tile_pool(name="attsmall", bufs=3 * KT + 3) as smallpool,
            tc.tile_pool(name="attwork", bufs=4) as workpool,
            tc.tile_pool(name="scratch", bufs=3) as scrpool,
            tc.tile_pool(name="psum_att", bufs=1, space="PSUM") as pa,
            tc.tile_pool(name="psum_mm", bufs=3, space="PSUM") as pm,
        ):
            # ---- constants ----
            wrb_t = cpool.tile([128, D], F32)
            nc.sync.dma_start(wrb_t[:], wrb[:])
            ones_f32 = cpool.tile([128, 128], F32, tag="ones_f32")
            nc.vector.memset(ones_f32[:], 1.0)
            ones_t = cpool.tile([128, 128], F32R, tag="ones")
            nc.vector.tensor_copy(ones_t[:], ones_f32[:])
            ln2_t = cpool.tile([128, 1], F32, tag="ln2")
            nc.vector.memset(ln2_t[:], float(np.log(2.0)))
            biases_t = cpool.tile([128, 51], F32, tag="biases")
            _BOFF = {"ib1": 0, "ib2": 8, "ub1": 12, "ub2": 28, "mb1": 36,
                     "mb2": 44, "mb3": 48, "mb4": 50}

            def bias_ap(name, m):
                return biases_t[:, _BOFF[name] + m:_BOFF[name] + m + 1]

            # ---- weight streaming ----
            wtiles = {}

            def fetch_weights(name, pool=None, tag="w"):
                pool = pool or wpool
                K, M = _LAYERS[name]
                ksub = _KSUB[name]
                chunks = []
                for c in range(K // (128 * ksub)):
                    t = pool.tile([128, ksub, M], F32R, tag=tag,
                                  name=f"w_{name}_{c}")
                    src = w_dram[name][c * 128 * ksub:(c + 1) * 128 * ksub, :]
                    nc.sync.dma_start(
                        t[:].bitcast(F32R),
                        src.rearrange("(a p) m -> p a m", p=128).bitcast(F32R))
                    chunks.append(t)
                wtiles[name] = (chunks, ksub)

            def layer_lhsT(name, kchunk, m):
                chunks, ksub = wtiles[name]
                t = chunks[kchunk // ksub]
                msz = min(128, _LAYERS[name][1])
                return t[:, kchunk % ksub, ts(m, msz)]

            # ---- dense layer: out_T[m] = relu(W.T @ x_T + b) ----
            def dense(name, x_chunks, bias_name, tag, share_tag=None):
                K, M = _LAYERS[name]
                nk, nm = K // 128, M // 128
                out_t = apool.tile([128, nm, BL], F32R,
                                   tag=share_tag or tag, name=f"act_{tag}")
                for m in range(nm):
                    ps = pm.tile([128, BL], F32, tag="mm", name=f"ps_{tag}{m}")
                    for k in range(nk):
                        nc.tensor.matmul(
                            ps[:], layer_lhsT(name, k, m), x_chunks[k],
                            start=(k == 0), stop=(k == nk - 1))
                    nc.scalar.activation(
                        out_t[:, m, :], ps[:], AF.Relu,
                        bias=bias_ap(bias_name, m))
                return [out_t[:, m, :] for m in range(nm)]

            # k-outer variant: weight chunks are consumed as they arrive, so
            # after the layer's last DMA byte only nm matmuls + drains remain.
            # Uses up to 8 PSUM banks (5 from the attention pool + 3 "mm").
            def dense_kouter(name, x_chunks, bias_name, tag, share_tag=None):
                K, M = _LAYERS[name]
                nk, nm = K // 128, M // 128
                assert nm <= 8
                out_t = apool.tile([128, nm, BL], F32R,
                                   tag=share_tag or tag, name=f"act_{tag}")
                ps_tags = [f"uf{i}" for i in range(4)] + ["s"]
                ps = []
                for m in range(nm):
                    if m < 5:
                        ps.append(pa.tile([128, BL], F32, tag=ps_tags[m],
                                          name=f"ko_{tag}{m}"))
                    else:
                        ps.append(pm.tile([128, BL], F32, tag="mm",
                                          name=f"ko_{tag}{m}"))
                for k in range(nk):
                    for m in range(nm):
                        nc.tensor.matmul(
                            ps[m][:], layer_lhsT(name, k, m), x_chunks[k],
                            start=(k == 0), stop=(k == nk - 1))
                for m in range(nm):
                    # alternate drains between ScalarE and VectorE so the
                    # layer-end drain doesn't serialize on one engine
                    if m % 2 == 0:
                        nc.scalar.activation(
                            out_t[:, m, :], ps[m][:], AF.Relu,
                            bias=bias_ap(bias_name, m))
                    else:
                        nc.vector.tensor_scalar(
                            out=out_t[:, m, :], in0=ps[m][:],
                            scalar1=bias_ap(bias_name, m), scalar2=0.0,
                            op0=ALU.add, op1=ALU.max)
                return [out_t[:, m, :] for m in range(nm)]

            # ---- attention phase (DMA-priority: rated/um first) ----
            uf_ps = [pa.tile([128, BL], F32, tag=f"uf{m}", name=f"uf_ps{m}")
                     for m in range(4)]
            s_ps = pa.tile([128, BL], F32, tag="s", name="s_ps")

            for g in range(NGRP):
                rated_t = spool.tile([128, GRP, D], F32R, tag="rated",
                                     name=f"rated{g}")
                nc.sync.dma_start(
                    rated_t[:].bitcast(F32R),
                    rated[g * GRP * 128:(g + 1) * GRP * 128, :]
                    .rearrange("(a p) d -> p a d", p=128).bitcast(F32R))
                um_t = spool.tile([128, GRP, BL], F32, tag="um",
                                  name=f"um{g}")
                nc.sync.dma_start(
                    um_t[:],
                    umT.rearrange("(p c) b -> p c b", c=KT)
                    [:, g * GRP:(g + 1) * GRP, :])

                for j in range(GRP):
                    k = g * GRP + j
                    # r_k[p] = sum_d rated[p,d]*wr[d]: DVE mul, ACT row-sum
                    prod = scrpool.tile([128, D], F32, tag="ttr",
                                        name=f"prod{k}")
                    nc.vector.tensor_mul(
                        prod[:], rated_t[:, j, :], wrb_t[:])
                    prod2 = scrpool.tile([128, D], F32, tag="ttr2",
                                         name=f"prod2_{k}")
                    r_k = smallpool.tile([128, 1], F32, tag="r", name=f"r{k}")
                    nc.scalar.activation(prod2[:], prod[:], AF.Copy,
                                         accum_out=r_k[:])
                    # e = exp(r); e2 = 2*exp(r) = exp(r + ln2)
                    e_k = smallpool.tile([128, 1], F32, tag="e", name=f"e{k}")
                    nc.scalar.activation(e_k[:], r_k[:], AF.Exp)
                    e2_k = smallpool.tile([128, 1], F32, tag="e2",
                                          name=f"e2{k}")
                    nc.scalar.activation(e2_k[:], r_k[:], AF.Exp,
                                         bias=ln2_t[:, 0:1])
                    # w_raw = um * e[i]
                    w_raw = workpool.tile([128, BL], F32R, tag="wraw",
                                          name=f"wraw{k}")
                    nc.vector.tensor_scalar(
                        out=w_raw[:], in0=um_t[:, j, :],
                        scalar1=e_k[:, 0:1], scalar2=None, op0=ALU.mult)
                    # mask*e = min(2e*um, e)
                    mask_e = workpool.tile([128, BL], F32R, tag="maske",
                                           name=f"maske{k}")
                    nc.vector.tensor_scalar(
                        out=mask_e[:], in0=um_t[:, j, :],
                        scalar1=e2_k[:, 0:1], scalar2=e_k[:, 0:1],
                        op0=ALU.mult, op1=ALU.min)
                    for m in range(4):
                        nc.tensor.matmul(
                            uf_ps[m][:],
                            rated_t[:, j, ts(m, 128)], w_raw[:],
                            start=(k == 0), stop=(k == KT - 1))
                    nc.tensor.matmul(
                        s_ps[:], ones_t[:], mask_e[:],
                        start=(k == 0), stop=(k == KT - 1))

            # ---- item tower (independent; scheduler overlaps with above)
            candT_t = apool.tile([128, 4, BL], F32R, tag="candT")
            nc.sync.dma_start(
                candT_t[:].bitcast(F32R),
                candT.rearrange("(a p) b -> p a b", p=128).bitcast(F32R))
            nc.sync.dma_start(biases_t[:], biases[:])
            fetch_weights("iw1")
            fetch_weights("iw2")
            item_h1 = dense("iw1", [candT_t[:, kk, :] for kk in range(4)],
                            "ib1", tag="item_h1")
            item_emb = dense("iw2", item_h1, "ib2", tag="item_emb")

            # tail-layer weights are tiny: fetch early into own slots so the
            # kernel tail never waits on DMA
            fetch_weights("mw3", pool=wtailpool, tag="mw3")
            fetch_weights("mw4", pool=wtailpool, tag="mw4")

            # ---- S -> 1/S (guarded), uf = uf_raw / S ----
            s_sb = cpool.tile([128, BL], F32, tag="s_sb")
            nc.vector.tensor_scalar(
                out=s_sb[:], in0=s_ps[:], scalar1=1e-30, scalar2=None,
                op0=ALU.max)
            recip = cpool.tile([128, BL], F32, tag="recip")
            nc.vector.reciprocal(recip[:], s_sb[:])

            uf_t = apool.tile([128, 4, BL], F32R, tag="uf_sb")  # shared w/ user_emb
            for m in range(4):
                nc.vector.tensor_tensor(
                    uf_t[:, m, :], uf_ps[m][:], recip[:], ALU.mult)
            uf_chunks = [uf_t[:, m, :] for m in range(4)]

            # ---- user tower + MLP head (weights in consumption order) ----
            fetch_weights("uw1")
            fetch_weights("uw2")
            user_h1 = dense("uw1", uf_chunks, "ub1", tag="user_h1")
            user_emb = dense_kouter("uw2", user_h1, "ub2", tag="user_emb",
                                    share_tag="uf_sb")

            fetch_weights("mw1")
            x_chunks = item_emb + user_emb
            a1 = dense_kouter("mw1", x_chunks, "mb1", tag="a1",
                              share_tag="item_h1")
            fetch_weights("mw2")
            a2 = dense_kouter("mw2", a1, "mb2", tag="a2",
                              share_tag="item_emb")
            a3 = dense("mw3", a2, "mb3", tag="a3", share_tag="candT")

            ps4 = pm.tile([128, BL], F32, tag="mm", name="ps4")
            for k in range(2):
                nc.tensor.matmul(ps4[:1, :], layer_lhsT("mw4", k, 0), a3[k],
                                 start=(k == 0), stop=(k == 1))
            out_sb = cpool.tile([1, BL], F32, tag="out_sb")
            nc.scalar.activation(out_sb[:1, :], ps4[:1, :], AF.Identity,
                                 bias=biases_t[0:1, 50:51])
            nc.sync.dma_start(out_dram[:].rearrange("(o b) -> o b", o=1),
                              out_sb[:1, :])

    nc.compile()
    return nc


def _merge_biases(ib1, ib2, ub1, ub2, mb1, mb2, mb3, mb4):
    f = np.float32
    cols = []
    for b, n in ((ib1, 8), (ib2, 4), (ub1, 16), (ub2, 8), (mb1, 8),
                 (mb2, 4), (mb3, 2)):
        cols.append(np.asarray(b, f).reshape(n, 128).T)
    mb4col = np.zeros((128, 1), f)
    mb4col[0, 0] = np.asarray(mb4, f).reshape(())
    cols.append(mb4col)
    return np.ascontiguousarray(np.concatenate(cols, axis=1))


def _prep_host(candidate_items, rated_items, user_matrix, att_w,
               iw1, ib1, iw2, ib2, uw1, ub1, uw2, ub2,
               mw1, mb1, mw2, mb2, mw3, mb3, mw4, mb4):
    """Shard + lay out inputs for the 8 cores."""
    f = np.float32
    asc = np.ascontiguousarray

    wr = np.asarray(att_w, f)[D:, 0]                       # (512,)
    wrb = asc(np.broadcast_to(wr[None, :], (128, D)))

    shared = {
        "rated": asc(np.asarray(rated_items, f)),
        "wrb": wrb,
        "iw1": asc(np.asarray(iw1, f)), "iw2": asc(np.asarray(iw2, f)),
        "uw1": asc(np.asarray(uw1, f)), "uw2": asc(np.asarray(uw2, f)),
        "mw1": asc(np.asarray(mw1, f)), "mw2": asc(np.asarray(mw2, f)),
        "mw3": asc(np.asarray(mw3, f)), "mw4": asc(np.asarray(mw4, f)),
        "biases": _merge_biases(ib1, ib2, ub1, ub2, mb1, mb2, mb3, mb4),
    }
    cand = np.asarray(candidate_items, f)
    um = np.asarray(user_matrix, f)
    in_maps = []
    for c in range(NCORES):
        sl = slice(c * BL, (c + 1) * BL)
        m = dict(shared)
        m["candT"] = asc(cand[sl].T)
        m["umT"] = asc(um[sl].T)
        in_maps.append(m)
    return in_maps


def run(inputs, trace=False, tmpdir=None):
    if "nc" not in _CACHE:
        _CACHE["nc"] = _build()
    nc = _CACHE["nc"]
    in_maps = _prep_host(**{k: v for k, v in inputs.items()
                            if k not in ("att_b",)})
    res = run_bass_kernel_spmd(nc, in_maps, core_ids=list(range(NCORES)),
                               trace=trace, tmpdir=tmpdir)
    out = np.concatenate([res.results[c]["out"] for c in range(NCORES)])
    return out.reshape(B, 1).astype(np.float32), res


def kernel(**inputs):
    out, _ = run(inputs, trace=False)
    return out
